# revision 27
# baseline (speedup 1.0000x reference)
"""Banded multi-head attention (band half-width 64) on 8 TRN2 NeuronCores.

Sharding: token-parallel. 8 cores = 4 batches x 2 token-halves of 1024
queries each.  Attention is banded (|i-j| <= 64), so each core only needs a
64-token halo of keys/values around its slice; QKV projections, banded
attention and the output projection all run locally with zero collectives.

On-chip layouts are feature-major (transposed) so every matmul runs fp16
operands (full PE rate, FWL weight loads) with fp32 PSUM accumulation.
Performance structure (vs the v1 kernel):
  - V projection starts as soon as the first wv/xv feature chunk lands
    (fi-outer accumulation over the first token-pair chains), so the PE
    never sits through the input DMA and the HAM clock-gate stays warm.
  - The ones rows of the attn@v lhsT (softmax denominator trick) are
    memset on-chip instead of DMA'd (saves 4.7MB of HBM traffic), and the
    v bias is folded into the output-projection bias on the host
    (rows of softmax sum to 1), so v lands bias-free via one ACT copy per
    token pair.
  - Scores for the two heads of a feature block run CONCURRENTLY on
    disjoint PE row groups (dk=64 -> rows 0:64 / 64:128), column-
    interleaved into shared [128,1024] PSUM tiles; edge kv strips are
    trimmed to their true 128-wide i-windows (2048 score cols per head).
  - One exp (ACT) and one mask multiply (GPSIMD) per 1024-col tile; q/k
    biases ride ACT, DVE does only reciprocal+scale, so no engine other
    than the PE is ever the bottleneck.
  - Output is stored fp16 (halves the output DMA; host casts to fp32).
"""

import math
import sys

sys.path.insert(0, "/opt/trn_rl_repo")

import numpy as np

import concourse.bacc as bacc
import concourse.mybir as mybir
import concourse.tile as tile
from concourse.bass_utils import run_bass_kernel_spmd

B, T, F = 4, 2048, 1024
H, DK = 16, 64
NCORES = 8
TLOC = 1024            # query tokens per core
PAD = 64               # band half-width = kv halo
KV = TLOC + 2 * PAD    # 1152 padded kv tokens per core
NT = KV // 128         # 9 kv strips

# per-strip i-window: strips 0 and 8 only cover 128 valid queries
IBASE = [0, 0, 128, 256, 384, 512, 640, 768, 896]
WID = [128, 256, 256, 256, 256, 256, 256, 256, 128]
# packing of strips into four 1024-col score tiles (cols 0:512 = even head,
# 512:1024 = odd head of the feature block); every tile column is written
TILE_STRIPS = [[(1, 0), (2, 256)],
               [(3, 0), (4, 256)],
               [(5, 0), (6, 256)],
               [(7, 0), (0, 256), (8, 384)]]
STRIP_LOC = {}
for _k, _lst in enumerate(TILE_STRIPS):
    for _t, _off in _lst:
        STRIP_LOC[_t] = (_k, _off)

# attn@v PSUM accumulation segments per 512-bank: (strip, dst, src, len)
def _bank_segs():
    segs = {0: [], 1: []}
    for t in range(NT):
        ib, w = IBASE[t], WID[t]
        if ib + w <= 512:
            segs[0].append((t, ib, 0, w))
        elif ib >= 512:
            segs[1].append((t, ib - 512, 0, w))
        else:
            segs[0].append((t, ib, 0, 512 - ib))
            segs[1].append((t, 0, 512 - ib, w - (512 - ib)))
    return segs

BANK_SEGS = _bank_segs()

F32 = mybir.dt.float32
F16 = mybir.dt.float16
AF = mybir.ActivationFunctionType

_cache = {}


def _build():
    nc = bacc.Bacc("TRN2", target_bir_lowering=False, debug=False,
                   num_devices=NCORES)
    xq = nc.dram_tensor("xq", [F, TLOC], F16, kind="ExternalInput").ap()
    xk = nc.dram_tensor("xk", [F, KV], F16, kind="ExternalInput").ap()
    xv = nc.dram_tensor("xv", [F, KV], F16, kind="ExternalInput").ap()
    wq = nc.dram_tensor("wq", [8, 128, F], F16, kind="ExternalInput").ap()
    wk = nc.dram_tensor("wk", [8, 128, F], F16, kind="ExternalInput").ap()
    wv = nc.dram_tensor("wv", [8, 128, F], F16, kind="ExternalInput").ap()
    wo = nc.dram_tensor("wo", [8, 128, F], F16, kind="ExternalInput").ap()
    bq = nc.dram_tensor("bq", [128, 8], F32, kind="ExternalInput").ap()
    bk = nc.dram_tensor("bk", [128, 8], F32, kind="ExternalInput").ap()
    bo = nc.dram_tensor("bo", [128, 8], F32, kind="ExternalInput").ap()
    msk = nc.dram_tensor("msk", [128, 4096], F16, kind="ExternalInput").ap()
    out = nc.dram_tensor("out", [F, TLOC], F16, kind="ExternalOutput").ap()

    with tile.TileContext(nc) as tc:
        with tc.tile_pool(name="pers", bufs=1) as pers, \
             tc.tile_pool(name="psum", bufs=1, space="PSUM") as psum:
            qTb = [pers.tile([128, TLOC], F16, tag=f"qT{ob}", name=f"qT{ob}")
                   for ob in range(8)]
            kTb = [pers.tile([128, KV], F16, tag=f"kT{ob}", name=f"kT{ob}")
                   for ob in range(8)]
            vau = pers.tile([128, NT * H * 128], F16, tag="vau")
            aTb = [pers.tile([128, TLOC], F16, tag=f"aT{fb}", name=f"aT{fb}")
                   for fb in range(8)]
            maskt = pers.tile([128, 4096], F16, tag="maskt")
            bqt = pers.tile([128, 8], F32, tag="bqt")
            bkt = pers.tile([128, 8], F32, tag="bkt")
            bot = pers.tile([128, 8], F32, tag="bot")

            va = vau[:].rearrange("p (t h e) -> p t h e", t=NT, h=H)
            # ones rows (softmax denominator) built on-chip; v rows are all
            # written by the V projection (padded kv tokens project to 0)
            nc.vector.memset(
                vau[:].rearrange("p (th e) -> p th e", e=128)[:, :, 0:64], 1.0)

            # warm up the PE clock gate (HAM) during the input-DMA dead time:
            # ~9us of garbage matmuls (operands are never-written SBUF, the
            # result is never read) so the first real V matmuls run at
            # 2.4GHz instead of spending 3.4us throttled at 1.2GHz
            warm = pers.tile([128, 512], F16, tag="warm")
            nc.gpsimd.memset(warm[:], 1.0)
            wps = [psum.tile([128, 512], F32, tag="mm", name=f"wps{i}",
                             bufs=2) for i in range(2)]
            for i in range(24):
                nc.tensor.matmul(wps[i % 2][:], warm[:, 0:128],
                                 warm[:, 0:512], start=True, stop=True)

            with tc.tile_pool(name="qkpool", bufs=1) as qkp:
                # ---------------- V projection ---------------------------
                # token-pair chains (tv) -> [128 tok, 1024 feat] PSUM tiles.
                # First two chains accumulate fi-outer so compute starts
                # after the first wv/xv chunk pair arrives.
                with tc.tile_pool(name="vpool", bufs=1) as vp:
                    # v inputs first, round-robin across all 3 DMA queues
                    # (each queue ~128GB/s; HBM saturates around 3 queues) so
                    # the V projection is never input-starved; everything
                    # else queues behind in need-by order.
                    # scalar's queue carries ONLY the early v chunks: DMA
                    # issues on an engine block its later compute via queue
                    # backpressure, and scalar must be free for exp/bias
                    # work. sync+gpsimd (no compute duties) carry the rest.
                    QS = [nc.sync, nc.scalar, nc.gpsimd]
                    wv_t, xv_t = [], []
                    for fi in range(8):
                        t = vp.tile([128, F], F16, tag=f"wv{fi}", name=f"wv{fi}")
                        wv_t.append(t)
                        t2 = vp.tile([128, KV], F16, tag=f"xv{fi}", name=f"xv{fi}")
                        xv_t.append(t2)
                        if fi == 0:
                            # halve the very first chunks so the first real
                            # matmul starts ~1.3us earlier
                            nc.sync.dma_start(t[:, 0:512], wv[0][:, 0:512])
                            nc.scalar.dma_start(t2[:, 0:512], xv[0:128, 0:512])
                            nc.sync.dma_start(t[:, 512:F], wv[0][:, 512:F])
                            nc.scalar.dma_start(t2[:, 512:KV],
                                                xv[0:128, 512:KV])
                            continue
                        QS[(2 * fi) % 3].dma_start(t[:], wv[fi])
                        QS[(2 * fi + 1) % 3].dma_start(
                            t2[:], xv[128 * fi:128 * (fi + 1), :])
                    nc.gpsimd.dma_start(bqt[:], bq[:])
                    nc.gpsimd.dma_start(bkt[:], bk[:])
                    nc.gpsimd.dma_start(bot[:], bo[:])
                    # x inputs next (full contraction needed by the first q/k
                    # chains), then the first two per-ob weight blocks, then
                    # masks; the remaining weight blocks trickle in need
                    # order.  wq/wk are repacked per OUTPUT block on the host
                    # so q_block(ob)/k_block(ob) each need only one 256KB
                    # block instead of the full 2MB weight.
                    xq_t, xk_t = [], []
                    for fi in range(8):
                        t2 = qkp.tile([128, TLOC], F16, tag=f"xq{fi}",
                                      name=f"xq{fi}")
                        xq_t.append(t2)
                        (nc.gpsimd if fi % 2 == 0 else nc.sync).dma_start(
                            t2[:], xq[128 * fi:128 * (fi + 1), :])
                    for fi in range(8):
                        t2 = qkp.tile([128, KV], F16, tag=f"xk{fi}",
                                      name=f"xk{fi}")
                        xk_t.append(t2)
                        (nc.gpsimd if fi % 2 == 0 else nc.sync).dma_start(
                            t2[:], xk[128 * fi:128 * (fi + 1), :])
                    wq_t = [qkp.tile([128, F], F16, tag=f"wq{ob}",
                                     name=f"wq{ob}") for ob in range(8)]
                    wk_t = [qkp.tile([128, F], F16, tag=f"wk{ob}",
                                     name=f"wk{ob}") for ob in range(8)]
                    for ob in range(2):
                        (nc.sync if ob == 0 else nc.gpsimd).dma_start(
                            wq_t[ob][:], wq[ob])
                        (nc.gpsimd if ob == 0 else nc.sync).dma_start(
                            wk_t[ob][:], wk[ob])
                    nc.sync.dma_start(maskt[:], msk[:])
                    for ob in range(2, 8):
                        nc.sync.dma_start(wq_t[ob][:], wq[ob])
                        nc.sync.dma_start(wk_t[ob][:], wk[ob])

                    def v_drain(tv, ps):
                        nc.vector.tensor_copy(
                            va[:, tv, :, 64:128], ps[:].rearrange(
                                "p (h e) -> p h e", e=64))

                    def v_drain_h(tv, oc, ps):
                        nc.vector.tensor_copy(
                            va[:, tv, 8 * oc:8 * (oc + 1), 64:128],
                            ps[:].rearrange("p (h e) -> p h e", e=64))

                    # chains 0-3: fi-outer over all 8 PSUM banks (compute
                    # starts as soon as the first wv/xv chunk pair lands)
                    ps02 = [psum.tile([128, 1024], F32, tag="sc",
                                      name=f"psv{c}", bufs=3) for c in range(3)]
                    ps3 = [psum.tile([128, 512], F32, tag="mm",
                                     name=f"psw{i}", bufs=2) for i in range(2)]
                    for fi in range(8):
                        for c in range(3):
                            for oc in range(2):
                                nc.tensor.matmul(
                                    ps02[c][:, 512 * oc:512 * (oc + 1)],
                                    xv_t[fi][:, 128 * c:128 * (c + 1)],
                                    wv_t[fi][:, 512 * oc:512 * (oc + 1)],
                                    start=(fi == 0), stop=(fi == 7))
                        for oc in range(2):
                            nc.tensor.matmul(
                                ps3[oc][:],
                                xv_t[fi][:, 384:512],
                                wv_t[fi][:, 512 * oc:512 * (oc + 1)],
                                start=(fi == 0), stop=(fi == 7))
                    for c in range(3):
                        v_drain(c, ps02[c])
                    for oc in range(2):
                        v_drain_h(3, oc, ps3[oc])
                    # chains 4..8: fi-inner, rotating
                    for tv in range(4, NT):
                        ps = psum.tile([128, 1024], F32, tag="sc",
                                       name=f"psv{tv}", bufs=3)
                        for fi in range(8):
                            for oc in range(2):
                                nc.tensor.matmul(
                                    ps[:, 512 * oc:512 * (oc + 1)],
                                    xv_t[fi][:, 128 * tv:128 * (tv + 1)],
                                    wv_t[fi][:, 512 * oc:512 * (oc + 1)],
                                    start=(fi == 0), stop=(fi == 7))
                        v_drain(tv, ps)

                # ------- q/k projections interleaved with attention -------
                with tc.tile_pool(name="ppool", bufs=1) as ppool, \
                     tc.tile_pool(name="lpool", bufs=1) as lpool, \
                     tc.tile_pool(name="opool", bufs=1) as opool, \
                     tc.tile_pool(name="wopool", bufs=1) as wop:
                    # prefetch output-projection weights (needed ~100us in)
                    wo_t = []
                    for fi in range(8):
                        t = wop.tile([128, F], F16, tag=f"wo{fi}",
                                     name=f"wo{fi}")
                        nc.sync.dma_start(t[:], wo[fi])
                        wo_t.append(t)

                    wqv = [t[:].rearrange("p (fi c) -> p fi c", c=128)
                           for t in wq_t]
                    wkv = [t[:].rearrange("p (fi c) -> p fi c", c=128)
                           for t in wk_t]

                    def q_block(ob):
                        for ch in range(2):
                            ps = psum.tile([128, 512], F32, tag="mm",
                                           name=f"psq{ob}{ch}", bufs=2)
                            for fi in range(8):
                                nc.tensor.matmul(
                                    ps[:], wqv[ob][:, fi, :],
                                    xq_t[fi][:, 512 * ch:512 * (ch + 1)],
                                    start=(fi == 0), stop=(fi == 7))
                            nc.scalar.activation(
                                qTb[ob][:, 512 * ch:512 * (ch + 1)], ps[:],
                                AF.Identity, bias=bqt[:, ob:ob + 1])

                    def k_block(ob):
                        for ch in range(3):
                            ps = psum.tile([128, 384], F32, tag="mm",
                                           name=f"psk{ob}{ch}", bufs=2)
                            for fi in range(8):
                                nc.tensor.matmul(
                                    ps[:], wkv[ob][:, fi, :],
                                    xk_t[fi][:, 384 * ch:384 * (ch + 1)],
                                    start=(fi == 0), stop=(fi == 7))
                            nc.scalar.activation(
                                kTb[ob][:, 384 * ch:384 * (ch + 1)], ps[:],
                                AF.Identity, bias=bkt[:, ob:ob + 1])

                    def scores_tiles(fb, ks):
                        # both heads of block fb, column-interleaved; the two
                        # heads' matmuls hit disjoint PE row groups (0:64 /
                        # 64:128) and run concurrently
                        ps = []
                        for k in ks:
                            sc = psum.tile([128, 1024], F32, tag="sc",
                                           name=f"sc{fb}_{k}", bufs=3)
                            for t, off in TILE_STRIPS[k]:
                                ib, w = IBASE[t], WID[t]
                                for hi in range(2):
                                    po = 64 * hi
                                    nc.tensor.matmul(
                                        sc[:, 512 * hi + off:512 * hi + off + w],
                                        kTb[fb][po:po + 64,
                                                128 * t:128 * (t + 1)],
                                        qTb[fb][po:po + 64, ib:ib + w],
                                        start=True, stop=True)
                            praw = ppool.tile([128, 1024], F16, tag="praw",
                                              name=f"praw{fb}_{k}", bufs=2)
                            nc.scalar.activation(praw[:], sc[:], AF.Exp)
                            p = ppool.tile([128, 1024], F16, tag="p",
                                           name=f"p{fb}_{k}", bufs=8)
                            nc.vector.tensor_mul(
                                p[:], praw[:],
                                maskt[:, 1024 * k:1024 * (k + 1)])
                            ps.append(p)
                        return ps

                    def attnv_block(h, p_tiles):
                        hi, fb = h % 2, h // 2
                        atl = psum.tile([128, 1024], F32, tag="sc",
                                        name=f"atl{h}", bufs=3)
                        for b in range(2):
                            segs = BANK_SEGS[b]
                            for idx, (t, do, so, ln) in enumerate(segs):
                                tk, off = STRIP_LOC[t]
                                nc.tensor.matmul(
                                    atl[:, 512 * b + do:512 * b + do + ln],
                                    va[:, t, h, :],
                                    p_tiles[tk][:, 512 * hi + off + so:
                                                512 * hi + off + so + ln],
                                    start=(idx == 0), stop=(idx == len(segs) - 1))
                        lbs = lpool.tile([64, 1024], F32, tag="lbs",
                                         name=f"lbs{h}", bufs=1)
                        nc.vector.reciprocal_approx_fast(
                            out=lbs[:], in_=atl[0:64, :])
                        nc.vector.tensor_mul(
                            aTb[fb][64 * hi:64 * hi + 64, :],
                            atl[64:128, :], lbs[:])

                    q_block(0)
                    k_block(0)
                    pend = {}
                    for fb in range(8):
                        pend[fb] = scores_tiles(fb, (0, 1))
                        if fb >= 1:
                            attnv_block(2 * fb - 2, pend[fb - 1])
                        if fb + 1 < 8:
                            q_block(fb + 1)
                        pend[fb] += scores_tiles(fb, (2, 3))
                        if fb >= 1:
                            attnv_block(2 * fb - 1, pend.pop(fb - 1))
                        if fb + 1 < 8:
                            k_block(fb + 1)
                    attnv_block(14, pend[7])
                    attnv_block(15, pend.pop(7))

                    # ---------------- output projection ----------------
                    # both 512-token halves of an output block share one
                    # 2-bank PSUM tile -> 3-deep rotation, one ACT drain and
                    # one 256KB output DMA per block
                    for ob in range(8):
                        if ob < 2:
                            # the sc-ring tiles are still draining the last
                            # attention heads; the mm tiles are free now
                            hps = [psum.tile([128, 512], F32, tag="mm",
                                             name=f"psoh{ob}{ch}", bufs=2)
                                   for ch in range(2)]
                            for fi in range(8):
                                for ch in range(2):
                                    nc.tensor.matmul(
                                        hps[ch][:],
                                        wo_t[fi][:, 128 * ob:128 * (ob + 1)],
                                        aTb[fi][:, 512 * ch:512 * (ch + 1)],
                                        start=(fi == 0), stop=(fi == 7))
                            for ch in range(2):
                                osh = opool.tile([128, 512], F16, tag="osh",
                                                 name=f"osh{ob}{ch}", bufs=2)
                                nc.scalar.activation(
                                    osh[:], hps[ch][:], AF.Identity,
                                    bias=bot[:, ob:ob + 1])
                                nc.sync.dma_start(
                                    out[128 * ob:128 * (ob + 1),
                                        512 * ch:512 * (ch + 1)], osh[:])
                            continue
                        ps = psum.tile([128, 1024], F32, tag="sc",
                                       name=f"pso{ob}", bufs=3)
                        for fi in range(8):
                            for ch in range(2):
                                nc.tensor.matmul(
                                    ps[:, 512 * ch:512 * (ch + 1)],
                                    wo_t[fi][:, 128 * ob:128 * (ob + 1)],
                                    aTb[fi][:, 512 * ch:512 * (ch + 1)],
                                    start=(fi == 0), stop=(fi == 7))
                        if ob < 7:
                            osb = opool.tile([128, 1024], F16, tag="osb",
                                             bufs=2)
                            nc.scalar.activation(
                                osb[:], ps[:], AF.Identity,
                                bias=bot[:, ob:ob + 1])
                            nc.sync.dma_start(
                                out[128 * ob:128 * (ob + 1), :], osb[:])
                        else:
                            # split the last block so its drain and output
                            # DMA pipeline (shorter kernel tail)
                            for ch in range(2):
                                osh = opool.tile([128, 512], F16, tag="osh",
                                                 name=f"osh{ch}", bufs=2)
                                nc.scalar.activation(
                                    osh[:], ps[:, 512 * ch:512 * (ch + 1)],
                                    AF.Identity, bias=bot[:, ob:ob + 1])
                                (nc.sync if ch == 0 else nc.gpsimd).dma_start(
                                    out[128 * ob:128 * (ob + 1),
                                        512 * ch:512 * (ch + 1)], osh[:])
    nc.compile()
    return nc


def _pack_ob(w, scale=1.0):
    # [o, f] weight -> [8, 128, F] fp16 row-tiles of W.T (cols = out features)
    wt = (np.asarray(w, np.float32) * scale).T        # [f, o]
    return np.ascontiguousarray(wt.reshape(8, 128, F)).astype(np.float16)


def _pack_outblk(w, scale=1.0):
    # [o, f] weight -> [8, 128, F] where block ob holds the weights for
    # output cols [128*ob, 128*(ob+1)) over the full contraction:
    # [ob, p, fi*128 + c] = W.T[128*fi + p, 128*ob + c]
    wt = (np.asarray(w, np.float32) * scale).T        # [f, o]
    wt = wt.reshape(8, 128, 8, 128).transpose(2, 1, 0, 3)
    return np.ascontiguousarray(wt.reshape(8, 128, F)).astype(np.float16)


def _host_masks(g0):
    m = np.zeros((128, 4096), np.float16)
    for k, lst in enumerate(TILE_STRIPS):
        for t, off in lst:
            ib, w = IBASE[t], WID[t]
            r = np.arange(128)[:, None]               # kv partition
            l = 128 * t + r                           # kv index in window
            i = ib + np.arange(w)[None, :]            # local query index
            jg = g0 - PAD + l                         # global key index
            valid = ((i >= l - 128) & (i <= l) &
                     (jg >= 0) & (jg < T)).astype(np.float16)
            for hi in range(2):
                m[:, 1024 * k + 512 * hi + off:
                  1024 * k + 512 * hi + off + w] = valid
    return m


def kernel(query, key, value, Wq, bq, Wk, bk, Wv, bv, Wo, bo, mask):
    query = np.asarray(query, np.float32)
    key = np.asarray(key, np.float32)
    value = np.asarray(value, np.float32)
    scale = 1.0 / math.sqrt(DK)

    if "nc" not in _cache:
        _cache["nc"] = _build()
    nc = _cache["nc"]

    # v bias folds into the output projection bias: rows of attn sum to 1
    bo2 = (np.asarray(bo, np.float32)
           + np.asarray(Wo, np.float32) @ np.asarray(bv, np.float32))
    shared = {
        "wq": _pack_outblk(Wq, scale),
        "wk": _pack_outblk(Wk),
        "wo": _pack_ob(Wo),
        "wv": _pack_ob(Wv),
        "bq": np.ascontiguousarray(
            (np.asarray(bq, np.float32) * scale).reshape(8, 128).T),
        "bk": np.ascontiguousarray(np.asarray(bk, np.float32).reshape(8, 128).T),
        "bo": np.ascontiguousarray(bo2.reshape(8, 128).T),
    }

    in_maps = []
    for c in range(NCORES):
        b, half = c // 2, c % 2
        g0 = half * TLOC
        lo, hi = max(0, g0 - PAD), min(T, g0 + TLOC + PAD)
        xkp = np.zeros((KV, F), np.float32)
        xvp = np.zeros((KV, F), np.float32)
        xkp[lo - (g0 - PAD):hi - (g0 - PAD)] = key[b, lo:hi]
        xvp[lo - (g0 - PAD):hi - (g0 - PAD)] = value[b, lo:hi]
        in_maps.append(dict(
            shared,
            xq=np.ascontiguousarray(query[b, g0:g0 + TLOC].T).astype(np.float16),
            xk=np.ascontiguousarray(xkp.T).astype(np.float16),
            xv=np.ascontiguousarray(xvp.T).astype(np.float16),
            msk=_host_masks(g0),
        ))

    res = run_bass_kernel_spmd(nc, in_maps, core_ids=list(range(NCORES)),
                               **_cache.get("run_kwargs", {}))
    _cache["last_result"] = res

    outp = np.empty((B, T, F), np.float32)
    for c in range(NCORES):
        b, half = c // 2, c % 2
        outp[b, half * TLOC:(half + 1) * TLOC] = \
            res.results[c]["out"].astype(np.float32).T
    return outp
